# revision 8
# baseline (speedup 1.0000x reference)
"""RoIAlignRotated Trainium2 kernel (v2).

Strategy: rois sharded across 8 NeuronCores (125 rois each); every core holds
a full replica of a precomputed "neighborhood table" in HBM: nb[b,y,x] = the
2x2 bilinear neighborhood [f(y,x), f(y,x+1), f(y+1,x), f(y+1,x+1)] of
channels-last features, fp16 (131072 rows x 2KB). One indirect-DMA descriptor
per sampling point fetches all four bilinear taps.

v2 layout per 128-bin tile (bin q = j*32 + b0):
  - ONE indirect gather op with a [128, 4] offset AP (512 descriptors):
    G[p=(b0*4+s), j, :] = nbhd row of sample s of bin (j,b0). SWDGE cost is
    ~1us fixed per op + ~sub-ns per descriptor, so merging 4 ops -> 1
    collapses the Q7 descriptor-generation stream (199us -> ~56us).
  - 16 matmuls (4 j-groups x 4 taps) with HOST-precomputed one-hot fp16
    stationaries [128, 32] (no on-device DVE expansion); group j accumulates
    into PSUM partitions 32j..32j+31 of a single [128, 256] f32 tile.
  - ONE ACT activation evacuates PSUM -> SBUF f16 (cast), ONE store per tile
    writes 128 contiguous f16 bin rows; host upcasts to f32.
"""

import os

import numpy as np

# Problem constants (hardcoded per contract; kernel.py must be self-contained).
B, C, H, W = 2, 256, 256, 256
N_ROIS = 1000
OH = OW = 7
GH = GW = 2
NSAMP = GH * GW                       # 4 sampling points per bin
SPATIAL_SCALE = 0.25
NCORES = 8

NROI_PC = N_ROIS // NCORES            # 125 rois per core
BINS_PC = NROI_PC * OH * OW           # 6125 output bins per core
NTILES = (BINS_PC + 127) // 128       # 48 tiles of 128 bins
BINS_PAD = NTILES * 128               # 6144
ROWS = B * H * W                      # 131072 neighborhood-table rows

_CACHE = {}
LAST_RESULTS = None  # BassKernelResults of the most recent run (for profiling)


def _build_bass():
    import concourse.bacc as bacc
    import concourse.bass as bass
    import concourse.mybir as mybir
    import concourse.tile as tile

    f32 = mybir.dt.float32
    f16 = mybir.dt.float16
    i32 = mybir.dt.int32

    nc = bacc.Bacc(
        "TRN2",
        target_bir_lowering=False,
        name="roialignrot",
        dynamic_dma_scratch_size=int(os.environ.get("ROI_DMA_SCRATCH", "16384")),
    )
    feat_d = nc.dram_tensor("feat", [ROWS, 4 * C], f16, kind="ExternalInput")
    idx_d = nc.dram_tensor("idx", [128, NTILES, NSAMP], i32, kind="ExternalInput")
    # host-expanded one-hot stationaries, partition-major: [p, t, 4j+nb, m]
    wts_d = nc.dram_tensor("wts", [128, NTILES, 16, 32], f16, kind="ExternalInput")
    out_d = nc.dram_tensor("out", [BINS_PAD, C], f16, kind="ExternalOutput")

    with tile.TileContext(nc) as tc:
        with (
            tc.tile_pool(name="const", bufs=1) as constp,
            tc.tile_pool(name="big", bufs=3) as bigp,
            tc.tile_pool(name="stage", bufs=4) as stagep,
            tc.tile_pool(name="psum", bufs=4, space="PSUM") as psump,
        ):
            # all offsets + stationaries up front so the gather stream never
            # waits on per-tile loads
            idx_all = constp.tile([128, NTILES, NSAMP], i32)
            nc.sync.dma_start(idx_all[:], idx_d[:])
            wts_all = constp.tile([128, NTILES, 16, 32], f16)
            nc.sync.dma_start(wts_all[:], wts_d[:])

            for t in range(NTILES):
                G = bigp.tile([128, NSAMP, 4 * C], f16, tag="g", name=f"g{t}")
                merge = int(os.environ.get("ROI_MERGE", "0"))
                if merge:
                    nc.gpsimd.indirect_dma_start(
                        out=G[:],
                        out_offset=None,
                        in_=feat_d[:],
                        in_offset=bass.IndirectOffsetOnAxis(
                            ap=idx_all[:, t, :], axis=0
                        ),
                    )
                else:
                    for j in range(NSAMP):
                        nc.gpsimd.indirect_dma_start(
                            out=G[:, j, :],
                            out_offset=None,
                            in_=feat_d[:],
                            in_offset=bass.IndirectOffsetOnAxis(
                                ap=idx_all[:, t, j:j + 1], axis=0
                            ),
                        )
                # PE tile_position allows PSUM base partitions {0, 32, 64}
                # only, so groups j=0,1 / j=2,3 target two [64, C] tiles
                ps0 = psump.tile([64, C], f32, tag="ps0")
                ps1 = psump.tile([64, C], f32, tag="ps1")
                for j in range(NSAMP):
                    ps = ps0 if j < 2 else ps1
                    for nb in range(4):
                        nc.tensor.matmul(
                            out=ps[32 * (j % 2):32 * (j % 2 + 1), :],
                            lhsT=wts_all[:, t, 4 * j + nb, :],
                            rhs=G[:, j, nb * C:(nb + 1) * C],
                            start=(nb == 0),
                            stop=(nb == 3),
                        )
                stage = stagep.tile([128, C], f16)
                nc.scalar.activation(
                    stage[0:64, :], ps0[:],
                    func=mybir.ActivationFunctionType.Copy,
                )
                nc.scalar.activation(
                    stage[64:128, :], ps1[:],
                    func=mybir.ActivationFunctionType.Copy,
                )
                nc.scalar.dma_start(out_d[t * 128:(t + 1) * 128, :], stage[:])

    nc.compile()
    return nc


def _get_nc():
    if "nc" not in _CACHE:
        _CACHE["nc"] = _build_bass()
    return _CACHE["nc"]


def _build_nbhd_table(features):
    """fp16 channels-last 2x2-neighborhood table [B*H*W, 4*C]."""
    f = features.transpose(0, 2, 3, 1).astype(np.float16)  # [B, H, W, C]
    nb = np.empty((B, H, W, 4, C), np.float16)
    xp = np.minimum(np.arange(W) + 1, W - 1)
    yp = np.minimum(np.arange(H) + 1, H - 1)
    nb[:, :, :, 0, :] = f
    nb[:, :, :, 1, :] = f[:, :, xp, :]
    nb[:, :, :, 2, :] = f[:, yp, :, :]
    nb[:, :, :, 3, :] = f[:, yp][:, :, xp]
    return nb.reshape(ROWS, 4 * C)


def _indices_weights(rois):
    """Per-bin sampling-point rows and folded weights, mirroring the
    reference math in float32.

    Returns idx [NBINS, 4] int32 and wts [NBINS, 4, 4] f32 (per-tap)."""
    f = np.float32
    b = rois[:, 0].astype(np.int32)
    cx = rois[:, 1] * f(SPATIAL_SCALE)
    cy = rois[:, 2] * f(SPATIAL_SCALE)
    rw = np.maximum(rois[:, 3] * f(SPATIAL_SCALE), f(0.0))
    rh = np.maximum(rois[:, 4] * f(SPATIAL_SCALE), f(0.0))
    theta = rois[:, 5]

    bin_h = rh / f(OH)
    bin_w = rw / f(OW)
    ph = np.arange(OH, dtype=f)
    pw = np.arange(OW, dtype=f)
    iy = (np.arange(GH, dtype=f) + f(0.5)) / f(GH)
    ix = (np.arange(GW, dtype=f) + f(0.5)) / f(GW)

    yy = (-rh / f(2.0))[:, None, None] + bin_h[:, None, None] * (
        ph[None, :, None] + iy[None, None, :]
    )  # [N, OH, GH]
    xx = (-rw / f(2.0))[:, None, None] + bin_w[:, None, None] * (
        pw[None, :, None] + ix[None, None, :]
    )  # [N, OW, GW]

    yyf = yy[:, :, None, :, None]  # [N, OH, 1, GH, 1]
    xxf = xx[:, None, :, None, :]  # [N, 1, OW, 1, GW]
    cosv = np.cos(theta)[:, None, None, None, None]
    sinv = np.sin(theta)[:, None, None, None, None]
    y = yyf * cosv - xxf * sinv + cy[:, None, None, None, None]  # [N,OH,OW,GH,GW]
    x = yyf * sinv + xxf * cosv + cx[:, None, None, None, None]

    valid = (y > f(-1.0)) & (y < f(H)) & (x > f(-1.0)) & (x < f(W))
    yc = np.clip(y, f(0.0), f(H - 1))
    xc = np.clip(x, f(0.0), f(W - 1))
    y0 = np.minimum(np.floor(yc).astype(np.int32), H - 1)
    x0 = np.minimum(np.floor(xc).astype(np.int32), W - 1)
    ly = yc - y0.astype(f)
    lx = xc - x0.astype(f)
    hy = f(1.0) - ly
    hx = f(1.0) - lx
    vm = valid.astype(f) * f(0.25)  # fold the mean over the GH*GW grid samples

    # tap weights; the table's clamped duplicate taps absorb the x1==x0 /
    # y1==y0 edge cases exactly
    w = np.stack([hy * hx, hy * lx, ly * hx, ly * lx], axis=-1) * vm[..., None]
    idx = b[:, None, None, None, None] * (H * W) + y0 * W + x0

    nbins = N_ROIS * OH * OW
    idx = idx.reshape(nbins, NSAMP).astype(np.int32)
    wts = w.reshape(nbins, NSAMP, 4).astype(f)
    return idx, wts


def _make_in_maps(features, rois):
    feat = _build_nbhd_table(features)
    idx_all, wts_all = _indices_weights(rois)
    # one-hot column pattern: partition p contributes to output column p//4
    p = np.arange(128)
    onehot = (p[:, None] // NSAMP == np.arange(32)[None, :])  # [128, 32]
    in_maps = []
    for core in range(NCORES):
        lo = core * BINS_PC
        hi = lo + BINS_PC
        idx_c = np.zeros((BINS_PAD, NSAMP), np.int32)
        wts_c = np.zeros((BINS_PAD, NSAMP, 4), np.float32)
        idx_c[:BINS_PC] = idx_all[lo:hi]
        wts_c[:BINS_PC] = wts_all[lo:hi]
        # bin q = j*32 + b0 within tile t; G partition p = b0*4 + s
        idx_t = idx_c.reshape(NTILES, NSAMP, 32, NSAMP)     # [t, j, b0, s]
        idx_t = idx_t.transpose(2, 3, 0, 1)                 # [b0, s, t, j]
        # stationaries: wts_e[t, 4j+nb, p=(b0,s), m] = w[bin(t,j,m), s, nb]
        # at m == b0, else 0
        wts_t = wts_c.reshape(NTILES, NSAMP, 32, NSAMP, 4)  # [t, j, m, s, nb]
        wts_t = wts_t.transpose(0, 1, 4, 2, 3)              # [t, j, nb, m, s]
        wts_e = np.zeros((NTILES, NSAMP, 4, 128, 32), np.float16)
        for s in range(NSAMP):
            # partition rows p = 4*b0 + s: value w[t, j, m=b0, s, nb] on the
            # diagonal b0 == m, zero elsewhere
            wts_e[:, :, :, s::4, :] = (
                wts_t[:, :, :, None, :, s] * onehot[s::4][None, None, None]
            ).astype(np.float16)
        # device layout: [p, t, k=(j,nb), m]
        wts_dev = wts_e.reshape(NTILES, 16, 128, 32).transpose(2, 0, 1, 3)
        in_maps.append(
            {
                "feat": feat,
                "idx": np.ascontiguousarray(idx_t.reshape(128, NTILES, NSAMP)),
                "wts": np.ascontiguousarray(wts_dev),
            }
        )
    return in_maps


def _unpack_out(res_out):
    o = res_out[:BINS_PC].astype(np.float32)
    return o.reshape(NROI_PC, OH, OW, C).transpose(0, 3, 1, 2)


def _ensure_ntff_hook():
    """bass_utils' trace=True path imports antenv.axon_hooks, which this
    image lacks — shim it (and install the libaxon NTFF hook) best-effort."""
    import sys
    import types

    if "antenv.axon_hooks" in sys.modules:
        return
    try:
        import antenv

        mod = types.ModuleType("antenv.axon_hooks")
        _hook = [None]
        mod.set_axon_ntff_profile_hook = lambda h: _hook.__setitem__(0, h)
        mod.get_axon_ntff_profile_hook = lambda: _hook[0]
        sys.modules["antenv.axon_hooks"] = mod
        antenv.axon_hooks = mod
        from trn_agent_boot.trn_boot import _ntff_profile_via_ctypes

        mod.set_axon_ntff_profile_hook(
            _ntff_profile_via_ctypes("/opt/axon/libaxon_pjrt.so")
        )
    except Exception:
        pass


def kernel(features, rois, out_w=7, out_h=7):
    global LAST_RESULTS
    from concourse.bass_utils import run_bass_kernel_spmd

    _ensure_ntff_hook()

    features = np.asarray(features, dtype=np.float32)
    rois = np.asarray(rois, dtype=np.float32)
    assert int(out_w) == OW and int(out_h) == OH
    assert features.shape == (B, C, H, W) and rois.shape == (N_ROIS, 6)

    in_maps = _make_in_maps(features, rois)
    nc = _get_nc()
    res = run_bass_kernel_spmd(
        nc,
        in_maps,
        core_ids=list(range(NCORES)),
        trace=bool(int(os.environ.get("ROI_TRACE", "0"))),
    )
    LAST_RESULTS = res
    outs = [_unpack_out(r["out"]) for r in res.results]
    return np.ascontiguousarray(np.concatenate(outs, axis=0))


# revision 10
# speedup vs baseline: 1.1048x; 1.1048x over previous
"""RoIAlignRotated Trainium2 kernel (v3).

Every core holds a full replica of a precomputed "neighborhood table" in HBM:
nb[b,y,x] = the 2x2 bilinear neighborhood [f(y,x), f(y,x+1), f(y+1,x),
f(y+1,x+1)] of channels-last features, fp16 (131072 rows x 2KB). One gather
descriptor per sampling point fetches all four bilinear taps.

v3 replaces per-tile indirect_dma_start (4 ops x 128 descriptors, ~1us fixed
SWDGE cost each => ~270us of Q7 serialization) with dma_gather: ONE op per
128-bin tile gathers all 512 sample rows (int16 indices), amortizing the
fixed cost 4x. dma_gather indices are int16, so rows are grouped into four
32768-row windows; bins are sharded across cores at BIN granularity sorted by
window (balanced, ~4% padding), and bins whose samples straddle two windows
are duplicated with complementary masked weights (host sums the partials).

Per 128-bin tile (bin q = j*32 + b0): G[p=(b0*4+s), j, :] = nbhd row of
sample s of bin (j,b0); 16 matmuls (4 j-groups x 4 taps) with host-built
one-hot fp16 stationaries [128, 32]; groups j=0,1 / j=2,3 accumulate into two
[64, C] PSUM tiles (PE base-partition rule); 2 ACT copies evacuate to an f16
stage; one 64KB store per tile. Host upcasts/accumulates f32 output.
"""

import os

import numpy as np

# Problem constants (hardcoded per contract; kernel.py must be self-contained).
B, C, H, W = 2, 256, 256, 256
N_ROIS = 1000
OH = OW = 7
GH = GW = 2
NSAMP = GH * GW                       # 4 sampling points per bin
SPATIAL_SCALE = 0.25
NCORES = 8

NBINS = N_ROIS * OH * OW              # 49000 output bins
ROWS = B * H * W                      # 131072 neighborhood-table rows
WIN = 32768                           # int16 index window (rows)
NWIN = ROWS // WIN                    # 4

_CACHE = {}
LAST_RESULTS = None  # BassKernelResults of the most recent run (for profiling)


def _build_bass(tiles_per_win):
    import concourse.bacc as bacc
    import concourse.library_config as library_config
    import concourse.mybir as mybir
    import concourse.tile as tile

    f32 = mybir.dt.float32
    f16 = mybir.dt.float16
    i16 = mybir.dt.int16

    nt = sum(tiles_per_win)
    nc = bacc.Bacc(
        "TRN2",
        target_bir_lowering=False,
        name="roialignrot",
    )
    feat_d = nc.dram_tensor("feat", [ROWS, 4 * C], f16, kind="ExternalInput")
    idx_d = nc.dram_tensor("idx", [128, nt, 32], i16, kind="ExternalInput")
    # host-expanded one-hot stationaries, partition-major: [p, t, 4j+nb, m]
    wts_d = nc.dram_tensor("wts", [128, nt, 16, 32], f16, kind="ExternalInput")
    out_d = nc.dram_tensor("out", [nt * 128, C], f16, kind="ExternalOutput")

    with tile.TileContext(nc) as tc:
        with (
            tc.tile_pool(name="const", bufs=1) as constp,
            tc.tile_pool(name="big", bufs=3) as bigp,
            tc.tile_pool(name="stage", bufs=4) as stagep,
            tc.tile_pool(name="psum", bufs=4, space="PSUM") as psump,
        ):
            nc.gpsimd.load_library(library_config.mlp)
            idx_all = constp.tile([128, nt, 32], i16)
            nc.sync.dma_start(idx_all[:], idx_d[:])
            wts_all = constp.tile([128, nt, 16, 32], f16)
            nc.sync.dma_start(wts_all[:], wts_d[:])

            t = 0
            for w in range(NWIN):
                for _ in range(tiles_per_win[w]):
                    G = bigp.tile([128, NSAMP, 4 * C], f16, tag="g", name=f"g{t}")
                    nc.gpsimd.dma_gather(
                        G[:],
                        feat_d[w * WIN:(w + 1) * WIN, :],
                        idx_all[:, t, :],
                        512,
                        512,
                        4 * C,
                    )
                    # PE tile_position allows PSUM base partitions {0, 32, 64}
                    # only, so groups j=0,1 / j=2,3 target two [64, C] tiles
                    ps0 = psump.tile([64, C], f32, tag="ps0")
                    ps1 = psump.tile([64, C], f32, tag="ps1")
                    for j in range(NSAMP):
                        ps = ps0 if j < 2 else ps1
                        for nb in range(4):
                            nc.tensor.matmul(
                                out=ps[32 * (j % 2):32 * (j % 2 + 1), :],
                                lhsT=wts_all[:, t, 4 * j + nb, :],
                                rhs=G[:, j, nb * C:(nb + 1) * C],
                                start=(nb == 0),
                                stop=(nb == 3),
                            )
                    stage = stagep.tile([128, C], f16)
                    nc.scalar.activation(
                        stage[0:64, :], ps0[:],
                        func=mybir.ActivationFunctionType.Copy,
                    )
                    nc.scalar.activation(
                        stage[64:128, :], ps1[:],
                        func=mybir.ActivationFunctionType.Copy,
                    )
                    nc.scalar.dma_start(out_d[t * 128:(t + 1) * 128, :], stage[:])
                    t += 1

    nc.compile()
    return nc


def _get_nc(tiles_per_win):
    key = tuple(tiles_per_win)
    if key not in _CACHE:
        _CACHE[key] = _build_bass(tiles_per_win)
    return _CACHE[key]


def _build_nbhd_table(features):
    """fp16 channels-last 2x2-neighborhood table [B*H*W, 4*C]."""
    f = features.transpose(0, 2, 3, 1).astype(np.float16)  # [B, H, W, C]
    nb = np.empty((B, H, W, 4, C), np.float16)
    xp = np.minimum(np.arange(W) + 1, W - 1)
    yp = np.minimum(np.arange(H) + 1, H - 1)
    nb[:, :, :, 0, :] = f
    nb[:, :, :, 1, :] = f[:, :, xp, :]
    nb[:, :, :, 2, :] = f[:, yp, :, :]
    nb[:, :, :, 3, :] = f[:, yp][:, :, xp]
    return nb.reshape(ROWS, 4 * C)


def _indices_weights(rois):
    """Per-bin sampling-point rows and folded weights, mirroring the
    reference math in float32.

    Returns idx [NBINS, 4] int32 and wts [NBINS, 4, 4] f32 (per-tap)."""
    f = np.float32
    b = rois[:, 0].astype(np.int32)
    cx = rois[:, 1] * f(SPATIAL_SCALE)
    cy = rois[:, 2] * f(SPATIAL_SCALE)
    rw = np.maximum(rois[:, 3] * f(SPATIAL_SCALE), f(0.0))
    rh = np.maximum(rois[:, 4] * f(SPATIAL_SCALE), f(0.0))
    theta = rois[:, 5]

    bin_h = rh / f(OH)
    bin_w = rw / f(OW)
    ph = np.arange(OH, dtype=f)
    pw = np.arange(OW, dtype=f)
    iy = (np.arange(GH, dtype=f) + f(0.5)) / f(GH)
    ix = (np.arange(GW, dtype=f) + f(0.5)) / f(GW)

    yy = (-rh / f(2.0))[:, None, None] + bin_h[:, None, None] * (
        ph[None, :, None] + iy[None, None, :]
    )  # [N, OH, GH]
    xx = (-rw / f(2.0))[:, None, None] + bin_w[:, None, None] * (
        pw[None, :, None] + ix[None, None, :]
    )  # [N, OW, GW]

    yyf = yy[:, :, None, :, None]  # [N, OH, 1, GH, 1]
    xxf = xx[:, None, :, None, :]  # [N, 1, OW, 1, GW]
    cosv = np.cos(theta)[:, None, None, None, None]
    sinv = np.sin(theta)[:, None, None, None, None]
    y = yyf * cosv - xxf * sinv + cy[:, None, None, None, None]  # [N,OH,OW,GH,GW]
    x = yyf * sinv + xxf * cosv + cx[:, None, None, None, None]

    valid = (y > f(-1.0)) & (y < f(H)) & (x > f(-1.0)) & (x < f(W))
    yc = np.clip(y, f(0.0), f(H - 1))
    xc = np.clip(x, f(0.0), f(W - 1))
    y0 = np.minimum(np.floor(yc).astype(np.int32), H - 1)
    x0 = np.minimum(np.floor(xc).astype(np.int32), W - 1)
    ly = yc - y0.astype(f)
    lx = xc - x0.astype(f)
    hy = f(1.0) - ly
    hx = f(1.0) - lx
    vm = valid.astype(f) * f(0.25)  # fold the mean over the GH*GW grid samples

    # tap weights; the table's clamped duplicate taps absorb the x1==x0 /
    # y1==y0 edge cases exactly
    w = np.stack([hy * hx, hy * lx, ly * hx, ly * lx], axis=-1) * vm[..., None]
    idx = b[:, None, None, None, None] * (H * W) + y0 * W + x0

    idx = idx.reshape(NBINS, NSAMP).astype(np.int32)
    wts = w.reshape(NBINS, NSAMP, 4).astype(f)
    return idx, wts


def _plan(rois):
    """Window-sorted entry pool, core split, and per-core device arrays.

    Returns (tiles_per_win, in_maps_meta) where in_maps_meta[core] =
    (idx16 [128, NT, 32], wts [128, NT, 16, 32], binmap [NT*128])."""
    idx_all, wts_all = _indices_weights(rois)
    win = idx_all // WIN  # [NBINS, 4]

    # entry pool: (window, idx4-relative, wts4-masked, bin)
    ent_w = []
    ent_idx = []
    ent_wts = []
    ent_bin = []
    wmin = win.min(axis=1)
    wmax = win.max(axis=1)
    for b in range(NBINS):
        for w in range(wmin[b], wmax[b] + 1):
            mask = win[b] == w
            if not mask.any():
                continue
            ent_w.append(w)
            ent_idx.append(np.where(mask, idx_all[b] - w * WIN, 0))
            ent_wts.append(wts_all[b] * mask[:, None])
            ent_bin.append(b)
    ent_w = np.array(ent_w)
    ent_idx = np.array(ent_idx, np.int32)
    ent_wts = np.array(ent_wts, np.float32)
    ent_bin = np.array(ent_bin, np.int32)

    order = np.argsort(ent_w, kind="stable")
    ent_w, ent_idx, ent_wts, ent_bin = (
        ent_w[order], ent_idx[order], ent_wts[order], ent_bin[order])

    counts = np.bincount(ent_w, minlength=NWIN)
    tiles_per_win = [int(np.ceil(c / (NCORES * 128))) for c in counts]
    nt = sum(tiles_per_win)
    cap_w = [tpw * 128 for tpw in tiles_per_win]

    # per-core, per-window entry slices (even split of each window's pool)
    starts = np.concatenate([[0], np.cumsum(counts)])
    onehot = (np.arange(128)[:, None] // NSAMP == np.arange(32)[None, :])

    in_maps_meta = []
    for core in range(NCORES):
        idx_t = np.zeros((nt, 128, NSAMP), np.int32)
        wts_t = np.zeros((nt, 128, NSAMP, 4), np.float32)
        binmap = np.full(nt * 128, -1, np.int32)
        t0 = 0
        for w in range(NWIN):
            n = counts[w]
            lo = starts[w] + min(core * (n // NCORES) + min(core, n % NCORES), n)
            hi = starts[w] + min((core + 1) * (n // NCORES) + min(core + 1, n % NCORES), n)
            sl = slice(lo, hi)
            k = hi - lo
            assert k <= cap_w[w]
            # scatter entries into this window's tiles, entry e -> (tile
            # t0 + e//128, slot e%128); slot q = j*32 + b0
            flat_idx = idx_t[t0:t0 + tiles_per_win[w]].reshape(-1, NSAMP)
            flat_wts = wts_t[t0:t0 + tiles_per_win[w]].reshape(-1, NSAMP, 4)
            flat_idx[:k] = ent_idx[sl]
            flat_wts[:k] = ent_wts[sl]
            binmap[t0 * 128:t0 * 128 + k] = ent_bin[sl]
            t0 += tiles_per_win[w]

        # device idx: op order i = j*128 + p, p = b0*4 + s, entry slot q=j*32+b0
        # idx_op[t, i] = idx_t[t, q(i), s(i)]
        q = idx_t.reshape(nt, NSAMP, 32, NSAMP)           # [t, j, b0, s]
        idx_op = q.transpose(0, 1, 2, 3).reshape(nt, NSAMP, 32, NSAMP)
        # i = j*128 + b0*4 + s -> order [j, b0, s]
        idx_op = idx_op.reshape(nt, 512)
        # int16 wrap: idx16[t, 16c+l, col] = idx_op[t, col*16+l]
        wrap = idx_op.reshape(nt, 32, 16).transpose(0, 2, 1)   # [t, l, col]
        idx16 = np.broadcast_to(
            wrap[:, None, :, :], (nt, 8, 16, 32)
        ).reshape(nt, 128, 32).transpose(1, 0, 2).astype(np.int16)

        # stationaries: wts_e[t, j, nb, p=(b0*4+s), m] = wts_t[t, q=(j,32+m), s, nb]
        # on the diagonal b0 == m
        wq = wts_t.reshape(nt, NSAMP, 32, NSAMP, 4)       # [t, j, m, s, nb]
        wq = wq.transpose(0, 1, 4, 2, 3)                  # [t, j, nb, m, s]
        wts_e = np.zeros((nt, NSAMP, 4, 128, 32), np.float16)
        for s in range(NSAMP):
            wts_e[:, :, :, s::4, :] = (
                wq[:, :, :, None, :, s] * onehot[s::4][None, None, None]
            ).astype(np.float16)
        wts_dev = wts_e.reshape(nt, 16, 128, 32).transpose(2, 0, 1, 3)

        in_maps_meta.append(
            (np.ascontiguousarray(idx16),
             np.ascontiguousarray(wts_dev),
             binmap)
        )
    return tiles_per_win, in_maps_meta


def _ensure_ntff_hook():
    """bass_utils' trace=True path imports antenv.axon_hooks, which this
    image lacks — shim it (and install the libaxon NTFF hook) best-effort."""
    import sys
    import types

    if "antenv.axon_hooks" in sys.modules:
        return
    try:
        import antenv

        mod = types.ModuleType("antenv.axon_hooks")
        _hook = [None]
        mod.set_axon_ntff_profile_hook = lambda h: _hook.__setitem__(0, h)
        mod.get_axon_ntff_profile_hook = lambda: _hook[0]
        sys.modules["antenv.axon_hooks"] = mod
        antenv.axon_hooks = mod
        from trn_agent_boot.trn_boot import _ntff_profile_via_ctypes

        mod.set_axon_ntff_profile_hook(
            _ntff_profile_via_ctypes("/opt/axon/libaxon_pjrt.so")
        )
    except Exception:
        pass


def kernel(features, rois, out_w=7, out_h=7):
    global LAST_RESULTS
    from concourse.bass_utils import run_bass_kernel_spmd

    _ensure_ntff_hook()

    features = np.asarray(features, dtype=np.float32)
    rois = np.asarray(rois, dtype=np.float32)
    assert int(out_w) == OW and int(out_h) == OH
    assert features.shape == (B, C, H, W) and rois.shape == (N_ROIS, 6)

    feat = _build_nbhd_table(features)
    tiles_per_win, metas = _plan(rois)
    in_maps = [
        {"feat": feat, "idx": idx16, "wts": wts_dev}
        for (idx16, wts_dev, _) in metas
    ]
    nc = _get_nc(tiles_per_win)
    res = run_bass_kernel_spmd(
        nc,
        in_maps,
        core_ids=list(range(NCORES)),
        trace=bool(int(os.environ.get("ROI_TRACE", "0"))),
    )
    LAST_RESULTS = res

    acc = np.zeros((NBINS, C), np.float32)
    for core in range(NCORES):
        out = res.results[core]["out"].astype(np.float32)   # [NT*128, C]
        binmap = metas[core][2]
        valid = binmap >= 0
        np.add.at(acc, binmap[valid], out[valid])
    out = acc.reshape(N_ROIS, OH, OW, C).transpose(0, 3, 1, 2)
    return np.ascontiguousarray(out)
